# revision 32
# baseline (speedup 1.0000x reference)
"""Trainium2 Bass kernel for a Deep Interest Evolution Network forward pass.

Computes, per batch row b (B=2048, T=200, E=H=128):
  GRU over hist_item_embd[b]  -> gru_out[t]          (interest extractor)
  attn[t] = (target@Wq+bq) . (gru_out[t]@Wk+bk) / sqrt(E)   (raw scores)
  AUGRU over (gru_out, attn) -> h_final[b]           (interest evolver)

Sharding: data-parallel over 8 NeuronCores, 256 batch rows per core.
On-chip layout is transposed: [feature=128 partitions, batch=256 free].
The host wrapper pre-transposes/casts inputs (fp16) so every DMA is
contiguous and no on-chip transposes are needed; fp32 PSUM accumulation
keeps the 200-step recurrences accurate (measured ~8e-3 rel err).

Two kernel variants:
  - _build_nc_v2 (fast path, used when all bias vectors are zero, which
    is what reference.setup_inputs() generates): all gates are computed
    as Tanh with host-halved weights (sigma(Wx) = 0.5*(tanh((W/2)x)+1)),
    the (tanh+1) factors fold into scalar_tensor_tensor ops, the
    attention score is pre-scaled by 0.5, updates use the precomputed
    form h' = (1-z) o n + z o h with the z-branches built off the serial
    chain on GPSIMD, n_pre is assembled in PSUM by an identity-weight
    matmul, and PSUM banks are hand-packed so accumulation-group windows
    open strictly one at a time per bank.  V2_NCHAINS splits the batch
    into independent recurrence chains to overlap per-step chain latency
    (dominated by cross-engine semaphore hops) with engine throughput.
  - _build_nc (general path, nonzero biases): the earlier
    sigmoid/tanh-based pipeline with bias columns riding in ScalarE
    activation APs.
Both run GRU step t interleaved with AUGRU step t-1 in one fused pass,
and use a TensorE ones-matmul to reduce+broadcast the raw attention
score in one shot.
"""

import numpy as np

import concourse.bass as bass
import concourse.bacc as bacc
import concourse.mybir as mybir
from concourse.tile import TileContext


B, T, E, H = 2048, 200, 128, 128
NCORES = 8
BL = B // NCORES  # 256 batch rows per core
CH = 20  # hist time-chunk per DMA (20 * 128KB = 1.25MB)
SCALE = 1.0 / float(np.sqrt(E))

F16 = mybir.dt.float16
F32 = mybir.dt.float32

LAST_RESULTS = None  # per-core output dicts of the last run (debugging)

_CACHED_NC = {}


def _build_nc_v2(repeat: int = 1, nchains: int = 1) -> bass.Bass:
    """All-tanh variant (requires all-zero biases, checked host-side).

    Sigmoid gates are computed as tanh with host-halved weights
    (sigma(Wx) = 0.5*(tanh((W/2)x)+1)), so the (t+1) factors fold into
    scalar_tensor_tensor ops and both updates use the precomputed form
      h' = (1-z) o n + z o h
    with (1-z) and z o h built off the serial chain.  The attention score
    is pre-scaled by 0.5 (qt, c) so za o a needs no extra 0.5.  n_pre is
    assembled in PSUM by an identity-weight matmul (inn + I @ rhn), which
    trades one DVE PSUM-mode op for a cheap PE op.  `nchains` splits the
    batch into independent recurrence chains whose instruction latencies
    overlap.

    PSUM is hand-packed, 3 banks per chain, so k=2 fits in 8 banks.  In
    every bank the accumulation-group windows open strictly one at a time
    (per-bank start-bit clears are only safe with no other open group):
      pb1 = [pre_r | hn' | na_pre | spare]
      pb2 = [pre_z | pre_ra | pre_za | bc]
      pb3 = [n_pre]   (exclusive: its window spans the body boundary)
    """
    nc = bacc.Bacc(
        "TRN2",
        target_bir_lowering=False,
        debug=False,
        num_devices=NCORES,
    )
    dt = F16
    W = BL // nchains

    histT = nc.declare_dram_parameter("histT", [E, T, BL], dt, isOutput=False)
    targT = nc.declare_dram_parameter("targT", [E, BL], dt, isOutput=False)
    w_ih = nc.declare_dram_parameter("w_ih", [E, 3 * H], dt, isOutput=False)
    w_hh = nc.declare_dram_parameter("w_hh", [H, 3 * H], dt, isOutput=False)
    wq = nc.declare_dram_parameter("wq", [E, H], dt, isOutput=False)
    wkT = nc.declare_dram_parameter("wkT", [H, H], dt, isOutput=False)
    bk_col = nc.declare_dram_parameter("bk_col", [H, 1], dt, isOutput=False)
    w_aug = nc.declare_dram_parameter("w_aug", [H, 6 * H], dt, isOutput=False)
    ident = nc.declare_dram_parameter("ident", [H, H], dt, isOutput=False)
    outT = nc.declare_dram_parameter("outT", [H, BL], F32, isOutput=True)

    Tanh = mybir.ActivationFunctionType.Tanh
    Sig = mybir.ActivationFunctionType.Sigmoid
    Ident = mybir.ActivationFunctionType.Identity
    ADD = mybir.AluOpType.add
    MULT = mybir.AluOpType.mult

    from contextlib import ExitStack

    with TileContext(nc) as tc:
        with (
            tc.tile_pool(name="const", bufs=1) as const,
            tc.tile_pool(name="state", bufs=3) as state,
            tc.tile_pool(name="work", bufs=3) as work,
            tc.tile_pool(name="hist", bufs=2) as histp,
        ):
            # ---- load constants ----
            sb_wih = const.tile([E, 3 * H], dt, tag="wih")
            nc.sync.dma_start(out=sb_wih, in_=w_ih[:, :])
            sb_whh = const.tile([H, 3 * H], dt, tag="whh")
            nc.sync.dma_start(out=sb_whh, in_=w_hh[:, :])
            sb_waug = const.tile([H, 6 * H], dt, tag="waug")
            nc.sync.dma_start(out=sb_waug, in_=w_aug[:, :])
            sb_wq = const.tile([E, H], dt, tag="wq")
            nc.sync.dma_start(out=sb_wq, in_=wq[:, :])
            sb_wkT = const.tile([H, H], dt, tag="wkT")
            nc.sync.dma_start(out=sb_wkT, in_=wkT[:, :])
            sb_bk = const.tile([H, 1], dt, tag="bk")
            nc.sync.dma_start(out=sb_bk, in_=bk_col[:, :])
            sb_id = const.tile([H, H], dt, tag="ident")
            nc.sync.dma_start(out=sb_id, in_=ident[:, :])
            sb_targ = const.tile([E, BL], dt, tag="targ")
            nc.sync.dma_start(out=sb_targ, in_=targT[:, :])
            ones = const.tile([H, H], dt, tag="ones")
            nc.vector.memset(ones, 1.0)

            # ---- attention setup (score pre-scaled by 0.5) ----
            qt_sb = const.tile([H, BL], dt, tag="qt")
            c_sb = const.tile([1, BL], dt, tag="csb")
            q_sb = const.tile([H, BL], dt, tag="qsb")
            with tc.tile_pool(name="psetup", bufs=1, space="PSUM") as psetup:
                ps_q = psetup.tile([H, BL], F32, tag="psq")
                nc.tensor.matmul(ps_q, sb_wq, sb_targ, start=True, stop=True)
                nc.scalar.activation(q_sb, ps_q, Ident)
                ps_qt = psetup.tile([H, BL], F32, tag="psqt")
                nc.tensor.matmul(ps_qt, sb_wkT, q_sb, start=True, stop=True)
                nc.scalar.activation(qt_sb, ps_qt, Ident, scale=SCALE)
                ps_c = psetup.tile([1, BL], F32, tag="psc")
                nc.tensor.matmul(ps_c, sb_bk, q_sb, start=True, stop=True)
                nc.scalar.activation(c_sb, ps_c, Ident, scale=SCALE)

            V = nc.vector
            P = nc.gpsimd

            for _rep in range(repeat):
                with ExitStack() as stk:
                    ch = []
                    for c in range(nchains):
                        pc = {}
                        p1 = stk.enter_context(
                            tc.tile_pool(name=f"pb1_{c}", bufs=1, space="PSUM")
                        )
                        p2 = stk.enter_context(
                            tc.tile_pool(name=f"pb2_{c}", bufs=1, space="PSUM")
                        )
                        p3 = stk.enter_context(
                            tc.tile_pool(name=f"pb3_{c}", bufs=1, space="PSUM")
                        )
                        if nchains == 1:
                            # z gets its own bank so its x-part can be
                            # prefetched (two concurrently-open accumulation
                            # windows must not share a bank).
                            p4 = stk.enter_context(
                                tc.tile_pool(name=f"pb4_{c}", bufs=1, space="PSUM")
                            )
                            pb1 = p1.tile([H, 3 * W], F32, tag=f"pb1_{c}")
                            pc["r"] = pb1[:, 0:W]
                            pc["hn"] = pb1[:, W : 2 * W]
                            pc["na"] = pb1[:, 2 * W : 3 * W]
                            pb4 = p4.tile([H, W], F32, tag=f"pb4_{c}")
                            pc["z"] = pb4
                            pc["rz"] = None
                        else:
                            # merged [r|z] so one tanh covers both gates; the
                            # z x-matmul then runs in-body (window stays
                            # sequential with r's).
                            pb1 = p1.tile([H, 4 * W], F32, tag=f"pb1_{c}")
                            pc["r"] = pb1[:, 0:W]
                            pc["z"] = pb1[:, W : 2 * W]
                            pc["rz"] = pb1[:, 0 : 2 * W]
                            pc["hn"] = pb1[:, 2 * W : 3 * W]
                            pc["na"] = pb1[:, 3 * W : 4 * W]
                        pb2 = p2.tile([H, 3 * W], F32, tag=f"pb2_{c}")
                        pb3 = p3.tile([H, W], F32, tag=f"pb3_{c}")
                        pc["ra"] = pb2[:, 0:W]
                        pc["za"] = pb2[:, W : 2 * W]
                        pc["raza"] = pb2[:, 0 : 2 * W]
                        pc["bc"] = pb2[:, 2 * W : 3 * W]
                        pc["ng"] = pb3
                        pc["sl"] = slice(c * W, (c + 1) * W)
                        ch.append(pc)

                    hist_sb = histp.tile([E, CH, BL], dt, tag="histc")
                    nc.sync.dma_start(out=hist_sb, in_=histT[:, 0:CH, :])

                    for c in range(nchains):
                        pc = ch[c]
                        S = state.tile([H, 2 * W], dt, tag=f"S{c}")
                        nc.vector.memset(S, 0.0)
                        pc["S"] = S
                        pc["m"] = None
                        x0 = hist_sb[:, 0, pc["sl"]]
                        nc.tensor.matmul(
                            pc["r"], sb_wih[:, 0:H], x0, start=True, stop=False
                        )
                        if nchains == 1:
                            nc.tensor.matmul(
                                pc["z"], sb_wih[:, H : 2 * H], x0,
                                start=True, stop=False,
                            )
                        nc.tensor.matmul(
                            pc["ng"], sb_wih[:, 2 * H : 3 * H], x0,
                            start=True, stop=False,
                        )

                    for t in range(T + 1):
                        if t + 1 < T and (t + 1) % CH == 0:
                            hist_sb = histp.tile([E, CH, BL], dt, tag="histc")
                            nc.sync.dma_start(
                                out=hist_sb, in_=histT[:, t + 1 : t + 1 + CH, :]
                            )
                        for c in range(nchains):
                            pc = ch[c]
                            sl = pc["sl"]
                            S_prev = pc["S"]
                            hg_prev = S_prev[:, 0:W]
                            hau_prev = S_prev[:, W:]
                            S = state.tile([H, 2 * W], dt, tag=f"S{c}")

                            # ---- PE: close GRU gate groups (h-side) ----
                            if t < T:
                                nc.tensor.matmul(
                                    pc["r"], sb_whh[:, 0:H], hg_prev,
                                    start=False, stop=True,
                                )
                                nc.tensor.matmul(
                                    pc["hn"], sb_whh[:, 2 * H : 3 * H],
                                    hg_prev, start=True, stop=True,
                                )
                                if nchains == 1:
                                    nc.tensor.matmul(
                                        pc["z"], sb_whh[:, H : 2 * H],
                                        hg_prev, start=False, stop=True,
                                    )
                                else:
                                    # z's x-part runs in-body (its bank is
                                    # shared with r; windows must sequence)
                                    nc.tensor.matmul(
                                        pc["z"], sb_wih[:, H : 2 * H],
                                        hist_sb[:, t % CH, sl],
                                        start=True, stop=False,
                                    )
                                    nc.tensor.matmul(
                                        pc["z"], sb_whh[:, H : 2 * H],
                                        hg_prev, start=False, stop=True,
                                    )
                            # ---- PE: AUGRU gates + score bcast ----
                            if t > 0:
                                nc.tensor.matmul(
                                    pc["ra"], sb_waug[:, 2 * H : 3 * H],
                                    hg_prev, start=True, stop=False,
                                )
                                nc.tensor.matmul(
                                    pc["ra"], sb_waug[:, 3 * H : 4 * H],
                                    hau_prev, start=False, stop=True,
                                )
                                nc.tensor.matmul(
                                    pc["za"], sb_waug[:, 0:H], hg_prev,
                                    start=True, stop=False,
                                )
                                nc.tensor.matmul(
                                    pc["za"], sb_waug[:, H : 2 * H], hau_prev,
                                    start=False, stop=True,
                                )

                            # ---- ACT: gates ----
                            if t < T:
                                if nchains == 1:
                                    t_r = work.tile([H, W], dt, tag=f"t_r{c}")
                                    nc.scalar.activation(t_r, pc["r"], Tanh)
                                    # z gate directly as sigmoid of the
                                    # (halved) pre-act: zz = sigma(2*pre_z')
                                    zz = work.tile([H, W], dt, tag=f"zz{c}")
                                    nc.scalar.activation(
                                        zz, pc["z"], Sig, scale=2.0
                                    )
                                    t_z = None
                                else:
                                    t_rz = work.tile(
                                        [H, 2 * W], dt, tag=f"t_rz{c}"
                                    )
                                    nc.scalar.activation(t_rz, pc["rz"], Tanh)
                                    t_r = t_rz[:, 0:W]
                                    t_z = t_rz[:, W:]
                            if t > 0:
                                # AUGRU gates directly as sigmoid of the
                                # (halved) pre-acts: sig = sigma(2*pre')
                                t_raza = work.tile(
                                    [H, 2 * W], dt, tag=f"t_raza{c}"
                                )
                                nc.scalar.activation(
                                    t_raza, pc["raza"], Sig, scale=2.0
                                )
                                t_ra = t_raza[:, 0:W]
                                t_za = t_raza[:, W:]
                                # bc emitted after t_raza so the gate tanh's
                                # conservative RAW set excludes the bc writes
                                nc.tensor.matmul(
                                    pc["bc"], ones, pc["m"],
                                    start=True, stop=False,
                                )
                                nc.tensor.matmul(
                                    pc["bc"], ones[0:1, :], c_sb[:, sl],
                                    start=False, stop=True,
                                )

                            # ---- DVE: candidate feeds ----
                            if t < T:
                                rhn = work.tile([H, W], dt, tag=f"rhn{c}")
                                V.scalar_tensor_tensor(
                                    rhn, t_r, 1.0, pc["hn"], op0=ADD, op1=MULT
                                )
                                # n_pre = inn + I @ rhn  (PE assembles in PSUM)
                                nc.tensor.matmul(
                                    pc["ng"], sb_id, rhn, start=False, stop=True
                                )
                            if t > 0:
                                rh2 = work.tile([H, W], dt, tag=f"rh2{c}")
                                V.tensor_mul(rh2, t_ra, hau_prev)

                            # ---- off-chain precomputes (z-forms) ----
                            if t < T:
                                if nchains > 1:
                                    zz = work.tile([H, W], dt, tag=f"zz{c}")
                                    P.tensor_scalar(
                                        zz, t_z, 0.5, 0.5, op0=MULT, op1=ADD
                                    )
                                wg = work.tile([H, W], dt, tag=f"wg{c}")
                                P.tensor_mul(wg, zz, hg_prev)
                                vg = work.tile([H, W], dt, tag=f"vg{c}")
                                P.tensor_scalar(
                                    vg, zz, -1.0, 1.0, op0=MULT, op1=ADD
                                )
                            if t > 0:
                                v2p = work.tile([H, W], dt, tag=f"v2p{c}")
                                V.tensor_mul(v2p, t_za, pc["bc"])
                                vb = work.tile([H, W], dt, tag=f"vb{c}")
                                V.tensor_scalar(
                                    vb, v2p, -1.0, 1.0, op0=MULT, op1=ADD
                                )
                                wb = work.tile([H, W], dt, tag=f"wb{c}")
                                V.tensor_mul(wb, vb, hau_prev)

                            # ---- PE: AUGRU candidate ----
                            if t > 0:
                                nc.tensor.matmul(
                                    pc["na"], sb_waug[:, 4 * H : 5 * H],
                                    hg_prev, start=True, stop=False,
                                )
                                nc.tensor.matmul(
                                    pc["na"], sb_waug[:, 5 * H : 6 * H], rh2,
                                    start=False, stop=True,
                                )

                            # ---- ACT: candidates ----
                            if t < T:
                                n = work.tile([H, W], dt, tag=f"n{c}")
                                nc.scalar.activation(n, pc["ng"], Tanh)
                            if t > 0:
                                na = work.tile([H, W], dt, tag=f"na{c}")
                                nc.scalar.activation(na, pc["na"], Tanh)

                            # ---- updates: h' = (1-z) o n + z o h ----
                            if t < T:
                                u = work.tile([H, W], dt, tag=f"u{c}")
                                V.tensor_mul(u, vg, n)
                                V.tensor_add(S[:, 0:W], u, wg)
                                if t == 0:
                                    nc.vector.tensor_copy(S[:, W:], S[:, 0:W])
                            if t > 0:
                                u2 = work.tile([H, W], dt, tag=f"u2{c}")
                                V.tensor_mul(u2, v2p, na)
                                V.tensor_add(S[:, W:], u2, wb)

                            # ---- m for next body's score ----
                            if t < T:
                                m = work.tile([H, W], dt, tag=f"m{c}")
                                P.tensor_mul(m, S[:, 0:W], qt_sb[:, sl])
                                pc["m"] = m

                            # ---- prefetch x-side for t+1 ----
                            if t + 1 < T:
                                x_nxt = hist_sb[:, (t + 1) % CH, sl]
                                nc.tensor.matmul(
                                    pc["r"], sb_wih[:, 0:H], x_nxt,
                                    start=True, stop=False,
                                )
                                if nchains == 1:
                                    nc.tensor.matmul(
                                        pc["z"], sb_wih[:, H : 2 * H], x_nxt,
                                        start=True, stop=False,
                                    )
                                nc.tensor.matmul(
                                    pc["ng"], sb_wih[:, 2 * H : 3 * H], x_nxt,
                                    start=True, stop=False,
                                )
                            pc["S"] = S

                    # ---- write result: h_au(T-1) per chain ----
                    for c in range(nchains):
                        out_sb = state.tile([H, W], F32, tag=f"out{c}")
                        nc.vector.tensor_copy(out_sb, ch[c]["S"][:, W:])
                        nc.sync.dma_start(
                            out=outT[:, ch[c]["sl"]], in_=out_sb
                        )

    nc.compile()
    return nc


def _build_nc(repeat: int = 1, use_pool: bool = True, pair_sig: bool = True) -> bass.Bass:
    nc = bacc.Bacc(
        "TRN2",
        target_bir_lowering=False,
        debug=False,
        num_devices=NCORES,
    )
    dt = F16
    ENG_M = nc.gpsimd if use_pool else nc.vector   # score elementwise
    ENG_Z = nc.gpsimd if use_pool else nc.vector   # 1-z precompute
    ENG_V = nc.gpsimd if use_pool else nc.vector   # z*h precompute

    histT = nc.declare_dram_parameter("histT", [E, T, BL], dt, isOutput=False)
    targT = nc.declare_dram_parameter("targT", [E, BL], dt, isOutput=False)
    w_ih = nc.declare_dram_parameter("w_ih", [E, 3 * H], dt, isOutput=False)
    w_hh = nc.declare_dram_parameter("w_hh", [H, 3 * H], dt, isOutput=False)
    wq = nc.declare_dram_parameter("wq", [E, H], dt, isOutput=False)
    wkT = nc.declare_dram_parameter("wkT", [H, H], dt, isOutput=False)
    bk_col = nc.declare_dram_parameter("bk_col", [H, 1], dt, isOutput=False)
    # AUGRU weights, split into x-half and h-half (natural lhsT layout)
    w_aug = nc.declare_dram_parameter("w_aug", [H, 6 * H], dt, isOutput=False)
    # fp32 per-partition bias columns:
    # [b_r_comb, b_z_comb, b_in, b_hn, bq, bz, br, bn]
    biases = nc.declare_dram_parameter("biases", [H, 8], F32, isOutput=False)
    outT = nc.declare_dram_parameter("outT", [H, BL], F32, isOutput=True)

    Sig = mybir.ActivationFunctionType.Sigmoid
    Tanh = mybir.ActivationFunctionType.Tanh
    Ident = mybir.ActivationFunctionType.Identity
    ADD = mybir.AluOpType.add
    MULT = mybir.AluOpType.mult

    with TileContext(nc) as tc:
        with (
            tc.tile_pool(name="const", bufs=1) as const,
            tc.tile_pool(name="state", bufs=3) as state,
            tc.tile_pool(name="work", bufs=3) as work,
            tc.tile_pool(name="hist", bufs=2) as histp,
        ):
            # ---- load constants ----
            sb_wih = const.tile([E, 3 * H], dt, tag="wih")
            nc.sync.dma_start(out=sb_wih, in_=w_ih[:, :])
            sb_whh = const.tile([H, 3 * H], dt, tag="whh")
            nc.sync.dma_start(out=sb_whh, in_=w_hh[:, :])
            sb_waug = const.tile([H, 6 * H], dt, tag="waug")
            nc.sync.dma_start(out=sb_waug, in_=w_aug[:, :])
            sb_wq = const.tile([E, H], dt, tag="wq")
            nc.sync.dma_start(out=sb_wq, in_=wq[:, :])
            sb_wkT = const.tile([H, H], dt, tag="wkT")
            nc.sync.dma_start(out=sb_wkT, in_=wkT[:, :])
            sb_bk = const.tile([H, 1], dt, tag="bk")
            nc.sync.dma_start(out=sb_bk, in_=bk_col[:, :])
            sb_bias = const.tile([H, 8], F32, tag="bias")
            nc.sync.dma_start(out=sb_bias, in_=biases[:, :])
            b_r = sb_bias[:, 0:1]
            b_z = sb_bias[:, 1:2]
            b_in = sb_bias[:, 2:3]
            b_hn = sb_bias[:, 3:4]
            b_q = sb_bias[:, 4:5]
            b_az = sb_bias[:, 5:6]
            b_ar = sb_bias[:, 6:7]
            b_an = sb_bias[:, 7:8]
            sb_targ = const.tile([E, BL], dt, tag="targ")
            nc.sync.dma_start(out=sb_targ, in_=targT[:, :])
            ones = const.tile([H, H], dt, tag="ones")
            nc.vector.memset(ones, 1.0)

            # ---- attention setup: qt = (Wk @ (Wq^T targ + bq)) * s ; c = q.bk * s
            qt_sb = const.tile([H, BL], dt, tag="qt")
            c_sb = const.tile([1, BL], dt, tag="csb")
            q_sb = const.tile([H, BL], dt, tag="qsb")
            with tc.tile_pool(name="psetup", bufs=1, space="PSUM") as psetup:
                ps_q = psetup.tile([H, BL], F32, tag="psq")
                nc.tensor.matmul(ps_q, sb_wq, sb_targ, start=True, stop=True)
                nc.scalar.activation(q_sb, ps_q, Ident, bias=b_q)
                ps_qt = psetup.tile([H, BL], F32, tag="psqt")
                nc.tensor.matmul(ps_qt, sb_wkT, q_sb, start=True, stop=True)
                nc.scalar.activation(qt_sb, ps_qt, Ident, scale=SCALE)
                ps_c = psetup.tile([1, BL], F32, tag="psc")
                nc.tensor.matmul(ps_c, sb_bk, q_sb, start=True, stop=True)
                nc.scalar.activation(c_sb, ps_c, Ident, scale=SCALE)

            for _rep in range(repeat):
                # ---- initial GRU hidden state ----
                h_g = state.tile([H, BL], dt, tag="h_g")
                nc.vector.memset(h_g, 0.0)
                h_au = None

                with (
                    tc.tile_pool(name="p_r", bufs=2, space="PSUM") as p_r,
                    tc.tile_pool(name="p_zar", bufs=2, space="PSUM") as p_zar,
                    tc.tile_pool(name="p_n2", bufs=1, space="PSUM") as p_n2,
                    tc.tile_pool(name="p_az", bufs=1, space="PSUM") as p_az,
                    tc.tile_pool(name="p_an", bufs=1, space="PSUM") as p_an,
                    tc.tile_pool(name="p_bc", bufs=1, space="PSUM") as p_bc,
                ):
                    hist_sb = None
                    # prologue: psum tiles + x-side matmuls for t=0
                    hist_sb = histp.tile([E, CH, BL], dt, tag="histc")
                    nc.sync.dma_start(out=hist_sb, in_=histT[:, 0:CH, :])
                    ps_r = p_r.tile([H, BL], F32, tag="psr")
                    ps_zar = p_zar.tile([H, 2 * BL], F32, tag="pszar")
                    ps_z = ps_zar[:, 0:BL]
                    ps_n = p_n2.tile([H, 2 * BL], F32, tag="n2")
                    nc.tensor.matmul(
                        ps_r, sb_wih[:, 0:H], hist_sb[:, 0, :],
                        start=True, stop=False,
                    )
                    nc.tensor.matmul(
                        ps_z, sb_wih[:, H : 2 * H], hist_sb[:, 0, :],
                        start=True, stop=False,
                    )
                    nc.tensor.matmul(
                        ps_n[:, 0:BL], sb_wih[:, 2 * H : 3 * H], hist_sb[:, 0, :],
                        start=True, stop=True,
                    )
                    for t in range(T + 1):
                        h_prev = h_g
                        hau_prev = h_au

                        # ---- PE: GRU(t) h-side matmuls into carried tiles ----
                        if t < T:
                            nc.tensor.matmul(
                                ps_r, sb_whh[:, 0:H], h_prev,
                                start=False, stop=True,
                            )
                            nc.tensor.matmul(
                                ps_z, sb_whh[:, H : 2 * H], h_prev,
                                start=False, stop=True,
                            )
                            nc.tensor.matmul(
                                ps_n[:, BL:], sb_whh[:, 2 * H : 3 * H], h_prev,
                                start=True, stop=True,
                            )

                        # ---- PE: AUGRU(t-1) z|r matmuls ----
                        # ar(t-1) shares a PSUM bank with z(t): the gh_z stop
                        # above closed the z accumulation group, so ar's
                        # start=True bank-bit clear cannot drop z contributions.
                        if t > 0:
                            nc.tensor.matmul(
                                ps_zar[:, BL:], sb_waug[:, 2 * H : 3 * H], h_prev,
                                start=True, stop=False,
                            )
                            nc.tensor.matmul(
                                ps_zar[:, BL:], sb_waug[:, 3 * H : 4 * H], hau_prev,
                                start=False, stop=True,
                            )
                            ps_a = p_az.tile([H, BL], F32, tag="az")
                            nc.tensor.matmul(
                                ps_a, sb_waug[:, 0:H], h_prev,
                                start=True, stop=False,
                            )
                            nc.tensor.matmul(
                                ps_a, sb_waug[:, H : 2 * H], hau_prev,
                                start=False, stop=True,
                            )

                        # ---- ACT priority order:
                        # sig_r, sig_z, sig_ar, tanh_n, sig_az, tanh_htl ----
                        if t < T:
                            r = work.tile([H, BL], dt, tag="r")
                            nc.scalar.activation(r, ps_r, Sig, bias=b_r)
                        if 0 < t < T and pair_sig:
                            zar = work.tile([H, 2 * BL], dt, tag="zar")
                            z = zar[:, 0:BL]
                            ar = zar[:, BL:]
                            nc.scalar.activation(zar, ps_zar, Sig, bias=b_z)
                        else:
                            if t < T:
                                z = work.tile([H, BL], dt, tag="z")
                                nc.scalar.activation(z, ps_z, Sig, bias=b_z)
                            if t > 0:
                                ar = work.tile([H, BL], dt, tag="ar")
                                nc.scalar.activation(
                                    ar, ps_zar[:, BL:], Sig, bias=b_ar
                                )

                        # ---- AUGRU(t-1) score bcast for step t-1 ----
                        if t > 0:
                            bc = p_bc.tile([H, BL], F32, tag="bc")
                            m = work.tile([H, BL], dt, tag="m")
                            ENG_M.tensor_mul(m, h_prev, qt_sb)
                            nc.tensor.matmul(bc, ones, m, start=True, stop=False)
                            nc.tensor.matmul(
                                bc, ones[0:1, :], c_sb, start=False, stop=True
                            )

                        # ---- GRU(t) candidate ----
                        if t < T:
                            g1 = work.tile([H, BL], dt, tag="g1")
                            nc.vector.tensor_scalar_add(g1, ps_n[:, BL:], b_hn)
                            g2 = work.tile([H, BL], dt, tag="g2")
                            nc.vector.tensor_scalar_add(g2, ps_n[:, 0:BL], b_in)
                            t1 = work.tile([H, BL], dt, tag="t1")
                            nc.vector.tensor_mul(t1, g1, r)
                            t2 = work.tile([H, BL], dt, tag="t2")
                            nc.vector.tensor_add(t2, g2, t1)
                            n = work.tile([H, BL], dt, tag="n")
                            nc.scalar.activation(n, t2, Tanh)
                        if t > 0:
                            az = work.tile([H, BL], dt, tag="az")
                            nc.scalar.activation(az, ps_a, Sig, bias=b_az)

                        # ---- AUGRU(t-1) candidate matmuls ----
                        if t > 0:
                            an = p_an.tile([H, BL], F32, tag="an")
                            rh = work.tile([H, BL], dt, tag="rh")
                            nc.vector.tensor_mul(rh, ar, hau_prev)
                            nc.tensor.matmul(
                                an, sb_waug[:, 4 * H : 5 * H], h_prev,
                                start=True, stop=False,
                            )
                            nc.tensor.matmul(
                                an, sb_waug[:, 5 * H : 6 * H], rh,
                                start=False, stop=True,
                            )

                        # ---- GRU(t) h' = (1-z)*n + z*h ----
                        if t < T:
                            zb = work.tile([H, BL], dt, tag="zb")
                            ENG_Z.tensor_scalar(
                                zb, z, -1.0, 1.0, op0=MULT, op1=ADD
                            )
                            v = work.tile([H, BL], dt, tag="v")
                            ENG_V.tensor_mul(v, z, h_prev)
                            u = work.tile([H, BL], dt, tag="u")
                            nc.vector.tensor_mul(u, zb, n)
                            h_new = state.tile([H, BL], dt, tag="h_g")
                            nc.vector.tensor_add(h_new, u, v)
                            h_g = h_new
                            if t == 0:
                                h_au = h_g

                        # ---- AUGRU(t-1) htl + za ----
                        if t > 0:
                            htl = work.tile([H, BL], dt, tag="htl")
                            nc.scalar.activation(htl, an, Tanh, bias=b_an)
                            za = work.tile([H, BL], dt, tag="za")
                            nc.vector.tensor_mul(za, az, bc)

                        # ---- AUGRU(t-1) h' = (1-za)*h + za*htl ----
                        if t > 0:
                            zab = work.tile([H, BL], dt, tag="zab")
                            ENG_Z.tensor_scalar(
                                zab, za, -1.0, 1.0, op0=MULT, op1=ADD
                            )
                            v2 = work.tile([H, BL], dt, tag="v2")
                            ENG_V.tensor_mul(v2, zab, hau_prev)
                            u2 = work.tile([H, BL], dt, tag="u2")
                            nc.vector.tensor_mul(u2, za, htl)
                            h_au_new = state.tile([H, BL], dt, tag="h_au")
                            nc.vector.tensor_add(h_au_new, u2, v2)
                            h_au = h_au_new

                        # ---- prefetch: x-side matmuls for step t+1 ----
                        if t + 1 < T:
                            if (t + 1) % CH == 0:
                                hist_sb = histp.tile([E, CH, BL], dt, tag="histc")
                                nc.sync.dma_start(
                                    out=hist_sb,
                                    in_=histT[:, t + 1 : t + 1 + CH, :],
                                )
                            x_nxt = hist_sb[:, (t + 1) % CH, :]
                            ps_r = p_r.tile([H, BL], F32, tag="psr")
                            ps_zar = p_zar.tile([H, 2 * BL], F32, tag="pszar")
                            ps_z = ps_zar[:, 0:BL]
                            ps_n = p_n2.tile([H, 2 * BL], F32, tag="n2")
                            nc.tensor.matmul(
                                ps_r, sb_wih[:, 0:H], x_nxt,
                                start=True, stop=False,
                            )
                            nc.tensor.matmul(
                                ps_z, sb_wih[:, H : 2 * H], x_nxt,
                                start=True, stop=False,
                            )
                            nc.tensor.matmul(
                                ps_n[:, 0:BL], sb_wih[:, 2 * H : 3 * H], x_nxt,
                                start=True, stop=True,
                            )


                # ---- write result ----
                out_sb = state.tile([H, BL], F32, tag="out")
                nc.vector.tensor_copy(out_sb, h_au)
                nc.sync.dma_start(out=outT[:, :], in_=out_sb)

    nc.compile()
    return nc


def _get_nc(pair_sig: bool = True):
    key = bool(pair_sig)
    if key not in _CACHED_NC:
        _CACHED_NC[key] = _build_nc(pair_sig=key)
    return _CACHED_NC[key]


# NOTE: nchains=2 currently produces wrong results on hardware (the list
# scheduler can reorder the in-body x-z matmul ahead of h-r within the
# shared pb1 bank, clearing the r-group's has_written bits mid-window).
# Keep at 1 unless that hazard is fixed (separate bank or explicit dep).
V2_NCHAINS = 1


def _get_nc_v2():
    key = ("v2", V2_NCHAINS)
    if key not in _CACHED_NC:
        _CACHED_NC[key] = _build_nc_v2(nchains=V2_NCHAINS)
    return _CACHED_NC[key]


def _prep_inputs_v2(
    target_item_embd, hist_item_embd, W_ih, W_hh, Wq, Wk, bk, Wz, Wr, Wn
):
    """Host prep for the all-tanh kernel (all biases must be zero)."""
    bf = np.float16
    # GRU: halve r,z blocks of both W_ih and W_hh; halve the n block of
    # W_hh only (the (t_r+1) fold); W_in stays full scale.
    w_ih = np.concatenate(
        [W_ih[:, 0:H] * 0.5, W_ih[:, H : 2 * H] * 0.5, W_ih[:, 2 * H :]], axis=1
    ).astype(bf)
    w_hh = np.concatenate(
        [W_hh[:, 0:H] * 0.5, W_hh[:, H : 2 * H] * 0.5, W_hh[:, 2 * H :] * 0.5],
        axis=1,
    ).astype(bf)
    # AUGRU layout: [Wr_x/2 | Wr_h/2 | Wz_x/2 | Wz_h/2 | Wn_x | Wn_h/2]
    # NOTE: kernel reads [2H:4H] for ra and [0:2H] for za, so za first.
    w_aug = np.concatenate(
        [
            Wz[:H] * 0.5, Wz[H:] * 0.5,   # za pre (slots 0,1)
            Wr[:H] * 0.5, Wr[H:] * 0.5,   # ra pre (slots 2,3)
            Wn[:H], Wn[H:],                # cand (slots 4,5)
        ],
        axis=1,
    ).astype(bf)
    shared = {
        "w_ih": np.ascontiguousarray(w_ih),
        "w_hh": np.ascontiguousarray(w_hh),
        "wq": np.ascontiguousarray(Wq.astype(bf)),
        "wkT": np.ascontiguousarray(Wk.T.astype(bf)),
        "bk_col": np.ascontiguousarray(bk.reshape(H, 1).astype(bf)),
        "w_aug": np.ascontiguousarray(w_aug),
        "ident": np.ascontiguousarray(np.eye(H, dtype=bf)),
    }
    in_maps = []
    for c in range(NCORES):
        sl = slice(c * BL, (c + 1) * BL)
        m = dict(shared)
        m["histT"] = np.ascontiguousarray(
            hist_item_embd[sl].transpose(2, 1, 0).astype(bf)
        )
        m["targT"] = np.ascontiguousarray(target_item_embd[sl].T.astype(bf))
        in_maps.append(m)
    return in_maps


def _prep_inputs(
    target_item_embd,
    hist_item_embd,
    W_ih,
    b_ih,
    W_hh,
    b_hh,
    Wq,
    bq,
    Wk,
    bk,
    Wz,
    bz,
    Wr,
    br,
    Wn,
    bn,
):
    """Host-side sharding/transposition. Returns (in_maps, pair_sig)."""
    bf = np.float16

    w_aug = np.concatenate(
        [Wz[:H], Wz[H:], Wr[:H], Wr[H:], Wn[:H], Wn[H:]], axis=1
    ).astype(bf)  # [H, 6H]
    b_r_comb = b_ih[0:H] + b_hh[0:H]
    b_z_comb = b_ih[H : 2 * H] + b_hh[H : 2 * H]
    biases = np.stack(
        [
            b_r_comb,
            b_z_comb,
            b_ih[2 * H : 3 * H],
            b_hh[2 * H : 3 * H],
            bq,
            bz,
            br,
            bn,
        ],
        axis=1,
    ).astype(np.float32)  # [H, 8]
    shared = {
        "w_ih": np.ascontiguousarray(W_ih.astype(bf)),
        "w_hh": np.ascontiguousarray(W_hh.astype(bf)),
        "wq": np.ascontiguousarray(Wq.astype(bf)),
        "wkT": np.ascontiguousarray(Wk.T.astype(bf)),
        "bk_col": np.ascontiguousarray(bk.reshape(H, 1).astype(bf)),
        "w_aug": np.ascontiguousarray(w_aug),
        "biases": np.ascontiguousarray(biases),
    }
    in_maps = []
    for c in range(NCORES):
        sl = slice(c * BL, (c + 1) * BL)
        m = dict(shared)
        m["histT"] = np.ascontiguousarray(
            hist_item_embd[sl].transpose(2, 1, 0).astype(bf)
        )  # [E, T, BL]
        m["targT"] = np.ascontiguousarray(target_item_embd[sl].T.astype(bf))
        in_maps.append(m)
    # The fused [z|ar] 512-wide sigmoid uses one per-partition bias vector
    # for both halves; only exact when those bias vectors coincide.
    pair_sig = bool(np.array_equal(b_z_comb, br))
    return in_maps, pair_sig


_CACHED_RUNNER = {}


def _get_runner(nc, key):
    """Cached jitted shard_map executable for `nc` (one per build variant).

    run_bass_kernel_spmd re-creates its jax.jit on every call, paying HLO
    re-compilation each time; this caches the executable so repeat
    kernel() calls only pay host prep + transfer + execution.
    """
    if key in _CACHED_RUNNER:
        return _CACHED_RUNNER[key]
    import jax
    from jax.experimental.shard_map import shard_map
    from jax.sharding import Mesh, PartitionSpec
    from concourse import bass2jax

    bass2jax.install_neuronx_cc_hook()
    partition_name = nc.partition_id_tensor.name if nc.partition_id_tensor else None
    in_names, out_names, out_avals = [], [], []
    for alloc in nc.m.functions[0].allocations:
        if not isinstance(alloc, mybir.MemoryLocationSet):
            continue
        name = alloc.memorylocations[0].name
        if alloc.kind == "ExternalInput":
            if name != partition_name:
                in_names.append(name)
        elif alloc.kind == "ExternalOutput":
            out_names.append(name)
            out_avals.append(
                jax.core.ShapedArray(
                    tuple(alloc.tensor_shape), mybir.dt.np(alloc.dtype)
                )
            )
    all_names = list(in_names) + list(out_names)
    if partition_name is not None:
        all_names.append(partition_name)

    def _body(*args):
        operands = list(args)
        if partition_name is not None:
            operands.append(bass2jax.partition_id_tensor())
        return tuple(
            bass2jax._bass_exec_p.bind(
                *operands,
                out_avals=tuple(out_avals),
                in_names=tuple(all_names),
                out_names=tuple(out_names),
                lowering_input_output_aliases=(),
                sim_require_finite=True,
                sim_require_nnan=True,
                nc=nc,
            )
        )

    devices = jax.devices()[:NCORES]
    mesh = Mesh(np.asarray(devices), ("core",))
    n_io = len(in_names) + len(out_names)
    fn = jax.jit(
        shard_map(
            _body,
            mesh=mesh,
            in_specs=(PartitionSpec("core"),) * n_io,
            out_specs=(PartitionSpec("core"),) * len(out_names),
            check_rep=False,
        ),
        keep_unused=True,
    )

    def run(in_maps):
        concat_in = [
            np.concatenate([np.asarray(in_maps[c][nm]) for c in range(NCORES)])
            for nm in in_names
        ]
        concat_zeros = [
            np.zeros((NCORES * a.shape[0], *a.shape[1:]), a.dtype)
            for a in out_avals
        ]
        outs = fn(*concat_in, *concat_zeros)
        return [
            {
                nm: np.asarray(outs[i]).reshape(NCORES, *out_avals[i].shape)[c]
                for i, nm in enumerate(out_names)
            }
            for c in range(NCORES)
        ]

    _CACHED_RUNNER[key] = run
    return run


def kernel(
    target_item_embd,
    hist_item_embd,
    W_ih,
    b_ih,
    W_hh,
    b_hh,
    Wq,
    bq,
    Wk,
    bk,
    Wv,
    bv,
    Wz,
    bz,
    Wr,
    br,
    Wn,
    bn,
):
    global LAST_RESULTS

    def f32(x):
        return np.asarray(x, np.float32)

    zero_bias = all(
        not np.any(f32(b)) for b in (b_ih, b_hh, bq, bk, bz, br, bn)
    )
    if zero_bias:
        in_maps = _prep_inputs_v2(
            f32(target_item_embd),
            f32(hist_item_embd),
            f32(W_ih),
            f32(W_hh),
            f32(Wq),
            f32(Wk),
            f32(bk),
            f32(Wz),
            f32(Wr),
            f32(Wn),
        )
        nc = _get_nc_v2()
        run = _get_runner(nc, "v2")
        results = run(in_maps)
        LAST_RESULTS = results
        return np.concatenate(
            [np.asarray(r["outT"], np.float32).T for r in results], axis=0
        )
    in_maps, pair_sig = _prep_inputs(
        f32(target_item_embd),
        f32(hist_item_embd),
        f32(W_ih),
        f32(b_ih),
        f32(W_hh),
        f32(b_hh),
        f32(Wq),
        f32(bq),
        f32(Wk),
        f32(bk),
        f32(Wz),
        f32(bz),
        f32(Wr),
        f32(br),
        f32(Wn),
        f32(bn),
    )
    nc = _get_nc(pair_sig)
    run = _get_runner(nc, pair_sig)
    results = run(in_maps)
    LAST_RESULTS = results
    out = np.concatenate(
        [np.asarray(r["outT"], np.float32).T for r in results], axis=0
    )
    return out

